# revision 13
# baseline (speedup 1.0000x reference)
"""HSTGNN adjacency-construction kernel for 8 Trainium2 NeuronCores.

Problem (per batch b):
  emb = [s; t]  (2144, 32)
  adj = emb @ emb.T
  ss  = adj[:N,:N] + 3*(n1@n2.T - n2@n1.T),  n_i = tanh(3*s@W_ssi.T)
  st  = adj[:N,N:] + (s@Wq_st.T+bq)@(t@Wk_st.T+bk).T
  ts  = adj[N:,:N] + (t@Wq_ts.T+bq)@(s@Wk_ts.T+bk).T
  tt  = adj[N:,N:]
  each block: x -> tanh(relu(x) / (GLOBAL max over batch of relu(x) + eps)),
  tt additionally upper-triangular masked.

The axon tunnel (~25-60 MB/s) dominates end-to-end time, so the design
minimizes host<->device bytes and launch count:
  - Batch-parallel: 2 batches per core; embT upload is 0.5 MB/core.
  - ONE launch: stacked-K matmuls produce every pre-activation block
    once for per-block maxima (DVE reduce_max -> [128,102] stats ->
    gpsimd partition_all_reduce -> 8-core AllReduce max collective ->
    reciprocal scales, all on device), then a second matmul pass applies
    ACT tanh(scale*x) and a DVE clamp+quantize to u8 in [0,255].
  - The u8 output is DMA'd through a bitcast view into a DRAM tensor
    declared uint32 (u8-declared outputs hit a pathological slow path
    in the tunnel's zero-donation upload).  Host dequantizes
    q * tanh(1)/255 and applies the tt triu mask.  u8 quantization adds
    ~0.5% l2 error (gate 2e-2).
  - A [128,1] probe through the same DVE convert detects whether
    f32->u8 conversion truncates or rounds, and sets the quantization
    bias on device accordingly.
"""

import math
import sys
import time

import numpy as np

sys.path.insert(0, "/opt/trn_rl_repo")

import concourse.bacc as bacc
import concourse.bass as bass
import concourse.bass_isa as bass_isa
import concourse.mybir as mybir
import concourse.tile as tile
from concourse.bass_utils import run_bass_kernel_spmd

F32 = mybir.dt.float32
F32R = mybir.dt.float32r
BF16 = mybir.dt.bfloat16
U8 = mybir.dt.uint8
U32 = mybir.dt.uint32
Act = mybir.ActivationFunctionType
Alu = mybir.AluOpType
AxX = mybir.AxisListType.X

B, N, T, D = 16, 2048, 96, 32
S = N + T          # 2144
NC = 8             # cores
BPC = B // NC      # batches per core
P = 128
NBAND = N // P     # 16 spatial row-bands
EPS = 1e-30
TANH1 = math.tanh(1.0)
QSCL = 127.0 / TANH1   # 7-bit: y in [-1, tanh(1)] -> y*QSCL in [-167, 127]
NG = S // 8            # 268 groups of 8 values -> 7 packed bytes
PKW = 7 * NG           # 1876 packed bytes per row

# stats column layout: block-contiguous so one tensor_reduce per block
#   ss: 0..63   (32*b + 2*r + h)
#   st: 64..95  (64 + 16*b + r)
#   ts: 96..99  (96 + 2*b + h)
#   tt: 100..101 (100 + b)
NSTAT = 102
_BLK = [(0, 64), (64, 32), (96, 4), (100, 2)]  # (col0, width) for ss/st/ts/tt

EXEC_NS = {}


def _emit_pack(nc, stagep, qt, qf, sfl, rows):
    """qt[rows, S] u8 ints 0..127 -> planar packed [rows, PKW] u8.
    b_k = v_k*2^(k+1) - 256*f_k + f_{k+1}, f_k = floor(v_k/2^(7-k))."""
    Alu = mybir.AluOpType
    nc.vector.tensor_scalar(qf[rows, :], qt[rows, :], 0.0, None, Alu.add)
    fu = stagep.tile([P, PKW], U8, tag="fu")
    a = stagep.tile([P, NG], F32, tag="pka")
    b2 = stagep.tile([P, NG], F32, tag="pkb")
    pk = stagep.tile([P, PKW], U8, tag="pk")
    for k in range(1, 8):
        nc.vector.tensor_scalar(
            fu[rows, (k - 1) * NG : k * NG], qf[rows, k::8],
            float(2.0 ** -(7 - k)), sfl[rows, 0:1], Alu.mult, Alu.subtract,
        )
    for k in range(7):
        if k == 0:
            nc.vector.tensor_scalar(a[rows, :], qf[rows, 0::8], 2.0, None, Alu.mult)
        else:
            nc.vector.tensor_scalar(
                a[rows, :], qf[rows, k::8], float(1 << (k + 1)), None, Alu.mult
            )
            nc.vector.tensor_scalar(
                b2[rows, :], fu[rows, (k - 1) * NG : k * NG], -256.0, None, Alu.mult
            )
            nc.vector.tensor_tensor(a[rows, :], a[rows, :], b2[rows, :], Alu.add)
        nc.vector.tensor_tensor(
            pk[rows, k * NG : (k + 1) * NG], a[rows, :],
            fu[rows, k * NG : (k + 1) * NG], Alu.add,
        )
    return pk



def _build():
    nc = bacc.Bacc("TRN2", target_bir_lowering=False, debug=False, num_devices=NC)

    embT_h = nc.dram_tensor("embT", [BPC, D, S], BF16, kind="ExternalInput")
    wp_h = nc.dram_tensor("Wpack", [D, 512], F32R, kind="ExternalInput")
    bias_h = nc.dram_tensor("biasp", [P, 4], F32, kind="ExternalInput")
    outq_h = nc.dram_tensor("outq", [BPC, S, PKW // 4], U32, kind="ExternalOutput")

    with tile.TileContext(nc) as tc:
        with (
            tc.tile_pool(name="const", bufs=1) as constp,
            tc.tile_pool(name="stage", bufs=3) as stagep,
            tc.tile_pool(name="psb", bufs=3, space="PSUM") as psb,
            tc.tile_pool(name="pss", bufs=2, space="PSUM") as pss,
            tc.tile_pool(name="dram", bufs=2, space="DRAM") as dramp,
        ):
            dma = nc.sync.dma_start
            outq_ap = outq_h.ap().bitcast(U8)  # [BPC, S, PKW] u8 view

            wp = constp.tile([D, 512], F32R, tag="wp")
            dma(wp[:, :], wp_h.ap()[:, :])
            biasp = constp.tile([P, 4], F32, tag="biasp")
            dma(biasp[:, :], bias_h.ap()[:, :])

            # ---- quant rounding probe: qb = 0.5 if u8 convert truncates ----
            prf = constp.tile([P, 1], F32, tag="prf")
            nc.vector.memset(prf[:, :], 1.5)
            prq = constp.tile([P, 1], U8, tag="prq")
            nc.vector.tensor_scalar(prq[:, :], prf[:, :], 0.0, None, Alu.add)
            prb = constp.tile([P, 1], F32, tag="prb")
            nc.vector.tensor_scalar(prb[:, :], prq[:, :], 0.0, None, Alu.add)
            # conv(1.5): 1 -> trunc -> qb=0.5 ; 2 -> round -> qb=0.0
            qb = constp.tile([P, 1], F32, tag="qb")
            nc.vector.tensor_scalar(qb[:, :], prb[:, :], -0.5, 1.0, Alu.mult, Alu.add)

            # floor bias for packing: 0.4921875 if convert rounds, 0 if truncates
            sfl = constp.tile([P, 1], F32, tag="sfl")
            nc.vector.tensor_scalar(
                sfl[:, :], qb[:, :], -0.984375, 0.4921875, Alu.mult, Alu.add
            )

            stats = constp.tile([P, NSTAT], F32, tag="stats")
            nc.vector.memset(stats[:, :], 0.0)

            # ---- phase 1: build U,V (kept resident) + all matmuls -> stats --
            UV = []
            for b in range(BPC):
                U = constp.tile([P, S], F32R, tag=f"U{b}")
                V = constp.tile([P, S], F32R, tag=f"V{b}")
                UV.append((U, V))
                ebf = stagep.tile([D, S], BF16, tag="ebf")
                dma(ebf[:, :], embT_h.ap()[b])
                nc.vector.tensor_copy(U[0:D, :], ebf[:, :])
                nc.vector.tensor_copy(V[0:D, :], ebf[:, :])

                # spatial linears: fill bands 1..3 of U and V
                for h in range(2):
                    hh = 1024 * h
                    for wofs, dst, bcol in ((0, U, 0), (128, V, 1)):
                        ps = psb.tile([P, 1024], F32, tag="ps")
                        for q in range(2):
                            c0 = hh + 512 * q
                            nc.tensor.matmul(
                                ps[:, 512 * q : 512 * q + 512],
                                wp[0:D, wofs : wofs + 128],
                                U[0:D, c0 : c0 + 512],
                                start=True,
                                stop=True,
                            )
                        nc.scalar.activation(
                            dst[32:64, hh : hh + 1024], ps[32:64, :], Act.Tanh
                        )
                        nc.scalar.activation(
                            dst[64:96, hh : hh + 1024], ps[64:96, :], Act.Tanh
                        )
                        nc.scalar.activation(
                            dst[96:128, hh : hh + 1024],
                            ps[96:128, :],
                            Act.Identity,
                            bias=biasp[96:128, bcol : bcol + 1],
                        )
                        if dst is U:
                            nc.vector.tensor_scalar_mul(
                                U[32:64, hh : hh + 1024],
                                U[32:64, hh : hh + 1024], 3.0,
                            )
                            nc.vector.tensor_scalar_mul(
                                U[64:96, hh : hh + 1024],
                                U[64:96, hh : hh + 1024], -3.0,
                            )

                # temporal linears: band 3 cols 2048:2144
                for wofs, dst, bcol in ((256, U, 2), (384, V, 3)):
                    psq = pss.tile([P, T], F32, tag="pst")
                    nc.tensor.matmul(
                        psq[:, :],
                        wp[0:D, wofs : wofs + 128],
                        U[0:D, N:S],
                        start=True,
                        stop=True,
                    )
                    nc.scalar.activation(
                        dst[96:128, N:S],
                        psq[96:128, :],
                        Act.Identity,
                        bias=biasp[96:128, bcol : bcol + 1],
                    )
                    # psq rows 32:96 are exactly 0 (zero weight cols):
                    # writes f32r zeros so K=128 st/ts skip bands 1-2
                    nc.scalar.activation(dst[32:64, N:S], psq[32:64, :], Act.Tanh)
                    nc.scalar.activation(dst[64:96, N:S], psq[64:96, :], Act.Tanh)

                # spatial row-bands: max over ss and st pre-activations
                for r in range(NBAND):
                    r0 = r * P
                    for h in range(2):
                        hh = 1024 * h
                        ps = psb.tile([P, 1024], F32, tag="ps")
                        for q in range(2):
                            c0 = hh + 512 * q
                            nc.tensor.matmul(
                                ps[:, 512 * q : 512 * q + 512],
                                U[0:96, r0 : r0 + P],
                                V[0:96, c0 : c0 + 512],
                                start=True,
                                stop=True,
                            )
                        c = 32 * b + 2 * r + h
                        nc.vector.tensor_reduce(
                            stats[:, c : c + 1], ps[:, :], AxX, Alu.max
                        )
                    pstt = pss.tile([P, T], F32, tag="pst")
                    nc.tensor.matmul(
                        pstt[:, :], U[:, r0 : r0 + P], V[:, N:S],
                        start=True, stop=True,
                    )
                    c = 64 + 16 * b + r
                    nc.vector.tensor_reduce(
                        stats[:, c : c + 1], pstt[:, :], AxX, Alu.max
                    )

                # temporal row-band (ts | tt)
                for h in range(2):
                    hh = 1024 * h
                    ps = psb.tile([P, 1024], F32, tag="ps")
                    for q in range(2):
                        c0 = hh + 512 * q
                        nc.tensor.matmul(
                            ps[0:T, 512 * q : 512 * q + 512],
                            U[:, N:S],
                            V[:, c0 : c0 + 512],
                            start=True, stop=True,
                        )
                    c = 96 + 2 * b + h
                    nc.vector.tensor_reduce(
                        stats[0:T, c : c + 1], ps[0:T, :], AxX, Alu.max
                    )
                pstt = pss.tile([P, T], F32, tag="pst")
                nc.tensor.matmul(
                    pstt[0:T, :], U[0:D, N:S], V[0:D, N:S], start=True, stop=True
                )
                c = 100 + b
                nc.vector.tensor_reduce(
                    stats[0:T, c : c + 1], pstt[0:T, :], AxX, Alu.max
                )

            # ---- global scales: partitions -> cores -> reciprocal ----------
            s4 = constp.tile([P, 4], F32, tag="s4")
            for j, (c0, w) in enumerate(_BLK):
                nc.vector.tensor_reduce(
                    s4[:, j : j + 1], stats[:, c0 : c0 + w], AxX, Alu.max
                )
            g4 = constp.tile([P, 4], F32, tag="g4")
            nc.gpsimd.partition_all_reduce(
                g4[:, :], s4[:, :], channels=P, reduce_op=bass_isa.ReduceOp.max
            )
            ib = dramp.tile([P, 4], F32, tag="ib")
            ob = dramp.tile([P, 4], F32, tag="ob")
            dma(ib[:, :], g4[:, :])
            nc.gpsimd.collective_compute(
                "AllReduce",
                Alu.max,
                replica_groups=[list(range(NC))],
                ins=[ib.opt()],
                outs=[ob.opt()],
            )
            gm = constp.tile([P, 4], F32, tag="gm")
            dma(gm[:, :], ob[:, :])
            scl = constp.tile([P, 4], F32, tag="scl")
            nc.vector.tensor_scalar_add(gm[:, :], gm[:, :], EPS)
            nc.vector.reciprocal(scl[:, :], gm[:, :])

            # ---- phase 2: recompute blocks, tanh(scale*x), quantize, DMA ---
            for b in range(BPC):
                U, V = UV[b]
                for r in range(NBAND):
                    r0 = r * P
                    stage = stagep.tile([P, S], F32, tag="stage")
                    qt = stagep.tile([P, S], U8, tag="qt")
                    for h in range(2):
                        hh = 1024 * h
                        ps = psb.tile([P, 1024], F32, tag="ps")
                        for q in range(2):
                            c0 = hh + 512 * q
                            nc.tensor.matmul(
                                ps[:, 512 * q : 512 * q + 512],
                                U[0:96, r0 : r0 + P],
                                V[0:96, c0 : c0 + 512],
                                start=True,
                                stop=True,
                            )
                        nc.scalar.activation(
                            stage[:, hh : hh + 1024],
                            ps[:, :],
                            Act.Tanh,
                            scale=scl[:, 0:1],
                        )
                    pstt = pss.tile([P, T], F32, tag="pst")
                    nc.tensor.matmul(
                        pstt[:, :], U[:, r0 : r0 + P], V[:, N:S],
                        start=True, stop=True,
                    )
                    nc.scalar.activation(
                        stage[:, N:S], pstt[:, :], Act.Tanh, scale=scl[:, 1:2]
                    )
                    # quantize: q = convert_u8(max(min(y*QSCL, 255), 0) + qb)
                    nc.vector.tensor_scalar(
                        stage[:, :], stage[:, :], QSCL, 127.0, Alu.mult, Alu.min
                    )
                    nc.vector.tensor_scalar(
                        qt[:, :], stage[:, :], 0.0, qb[:, 0:1], Alu.max, Alu.add
                    )
                    qf = stagep.tile([P, S], F32, tag="qf")
                    pk = _emit_pack(nc, stagep, qt, qf, sfl, slice(0, P))
                    dma(outq_ap[b, r0 : r0 + P, :], pk[:, :])

                # temporal row-band (ts | tt)
                stage = stagep.tile([P, S], F32, tag="stage")
                qt = stagep.tile([P, S], U8, tag="qt")
                for h in range(2):
                    hh = 1024 * h
                    ps = psb.tile([P, 1024], F32, tag="ps")
                    for q in range(2):
                        c0 = hh + 512 * q
                        nc.tensor.matmul(
                            ps[0:T, 512 * q : 512 * q + 512],
                            U[:, N:S],
                            V[:, c0 : c0 + 512],
                            start=True, stop=True,
                        )
                    nc.scalar.activation(
                        stage[0:T, hh : hh + 1024],
                        ps[0:T, :],
                        Act.Tanh,
                        scale=scl[0:T, 2:3],
                    )
                pstt = pss.tile([P, T], F32, tag="pst")
                nc.tensor.matmul(
                    pstt[0:T, :], U[0:D, N:S], V[0:D, N:S], start=True, stop=True
                )
                nc.scalar.activation(
                    stage[0:T, N:S], pstt[0:T, :], Act.Tanh, scale=scl[0:T, 3:4]
                )
                nc.vector.tensor_scalar(
                    stage[0:T, :], stage[0:T, :], QSCL, 127.0, Alu.mult, Alu.min
                )
                nc.vector.tensor_scalar(
                    qt[0:T, :], stage[0:T, :], 0.0, qb[0:T, 0:1], Alu.max, Alu.add
                )
                qf = stagep.tile([P, S], F32, tag="qf")
                pk = _emit_pack(nc, stagep, qt, qf, sfl, slice(0, T))
                dma(outq_ap[b, N:S, :], pk[0:T, :])

    nc.compile()
    return nc


_PROGS = {}


def _prog():
    if "one" not in _PROGS:
        _PROGS["one"] = _build()
    return _PROGS["one"]


def _host_pack(inputs):
    import ml_dtypes

    s = np.asarray(inputs["spatial_nodes"], dtype=np.float32)
    t = np.asarray(inputs["temporal_nodes"], dtype=np.float32)
    emb = np.concatenate([s, t], axis=1)                    # [B, S, D]
    embT = np.ascontiguousarray(
        emb.transpose(0, 2, 1).astype(ml_dtypes.bfloat16)
    )                                                       # [B, D, S] bf16

    wp = np.zeros((D, 512), dtype=np.float32)
    # U bands: 1 -> n1=tanh(3 s W1^T) (x3 later), 2 -> n2 (x-3 later), 3 -> q_st
    wp[:, 32:64] = (3.0 * np.asarray(inputs["W_ss1"])).T
    wp[:, 64:96] = (3.0 * np.asarray(inputs["W_ss2"])).T
    wp[:, 96:128] = np.asarray(inputs["Wq_st"]).T
    # V bands: 1 -> n2, 2 -> n1, 3 -> k_ts
    wp[:, 160:192] = (3.0 * np.asarray(inputs["W_ss2"])).T
    wp[:, 192:224] = (3.0 * np.asarray(inputs["W_ss1"])).T
    wp[:, 224:256] = np.asarray(inputs["Wk_ts"]).T
    # temporal: U band3 -> q_ts ; V band3 -> k_st
    wp[:, 352:384] = np.asarray(inputs["Wq_ts"]).T
    wp[:, 480:512] = np.asarray(inputs["Wk_st"]).T

    biasp = np.zeros((P, 4), dtype=np.float32)
    biasp[96:128, 0] = np.asarray(inputs["bq_st"])
    biasp[96:128, 1] = np.asarray(inputs["bk_ts"])
    biasp[96:128, 2] = np.asarray(inputs["bq_ts"])
    biasp[96:128, 3] = np.asarray(inputs["bk_st"])
    return embT, wp, biasp


def _run(nc, in_maps, profile):
    if profile:
        try:
            return run_bass_kernel_spmd(
                nc, in_maps, core_ids=list(range(NC)), trace=True
            )
        except Exception as e:  # no NTFF hook on this axon client
            print(f"trace unavailable ({type(e).__name__}: {e}); untraced", flush=True)
    return run_bass_kernel_spmd(nc, in_maps, core_ids=list(range(NC)), trace=False)


def kernel(profile=False, **inputs):
    tA = time.monotonic()
    embT, wp, biasp = _host_pack(inputs)

    common = {"Wpack": wp, "biasp": biasp}
    in_maps = [
        {"embT": embT[BPC * c : BPC * (c + 1)], **common} for c in range(NC)
    ]

    nc1 = _prog()
    t0 = time.monotonic()
    res = _run(nc1, in_maps, profile)
    t1 = time.monotonic()
    EXEC_NS["out"] = res.exec_time_ns
    EXEC_NS["out_wall"] = (t1 - t0) * 1e9

    dq = np.float32(TANH1 / 127.0)
    out = np.empty((B, S, S), dtype=np.float32)
    v = np.empty((BPC, S, NG, 8), dtype=np.uint8)
    for c in range(NC):
        pb = res.results[c]["outq"].view(np.uint8).reshape(BPC, S, 7, NG)
        pw = pb.astype(np.uint16)
        v[..., 0] = (pb[:, :, 0] >> 1)
        for k in range(1, 7):
            v[..., k] = (
                ((pw[:, :, k - 1] & ((1 << k) - 1)) << (7 - k))
                | (pw[:, :, k] >> (k + 1))
            ).astype(np.uint8)
        v[..., 7] = pb[:, :, 6] & 127
        np.multiply(
            v.reshape(BPC, S, S), dq,
            out=out[BPC * c : BPC * (c + 1)], casting="unsafe",
        )
    # tt block is upper-triangular (mask applied on host, post-dequant)
    tri = np.tril(np.ones((T, T), dtype=bool), k=-1)
    out[:, N:S, N:S][:, tri] = 0.0
    tB = time.monotonic()
    EXEC_NS["pack_wall"] = (t0 - tA) * 1e9
    EXEC_NS["post_wall"] = (tB - t1) * 1e9
    return out
